# revision 33
# baseline (speedup 1.0000x reference)
"""BiDAF attention (nn_BertBidafAttention) on 8 TRN2 NeuronCores.

Math (per batch, reference):
    cp = c @ W.T + b            [CL, H]
    s  = cp @ q.T               [CL, QL]
    s1 = softmax_q(s + qmask_bias)      (softmax over q)
    s2 = softmax_c(s + cmask_bias)      (softmax over c)
    a  = s1 @ q                 [CL, H]
    bv = (s1 @ s2.T) @ c = s1 @ (s2.T @ c)
    x  = [c, a, c*a, c*bv]      [CL, 4H]

Implementation notes:
  * fp16 end to end: the host casts c/q/W/b to fp16 (10-bit mantissa, same
    effective precision as f32r/TF32 which passes at 2.5e-3 rel err) and
    precomputes the additive mask biases (mask-1)*1000 so exp(masked-max)
    flushes to exactly 0.  fp16 matmuls run single-pass at full PE rate for
    any free size (fp32 runs two LOW/HIGH passes), and halve DMA + SBUF
    traffic.  PSUM accumulation stays fp32.
  * sT[q,c] = (W.T qT).T @ cT + rank-2 bias: the projection cost drops from
    c@W (604 MF) to W.T@qT (75 MF); the rank-2 matmul [qrow;1].T@[1;cbias]
    adds qrow[q] = b.q + qmask_bias and cbias[c] in one PE op.  Both
    softmaxes read the same biased logits: the per-q terms cancel in the
    softmax over q... (s2 is over c per q-row: per-q shift cancels; s1 is
    over q per c-column: the per-c cbias cancels).
  * layout transposes (c->cT, q->qT, sTb->s_nat, s2T->s2) run on the DMA
    XBAR transpose unit (InstDmaTransposeAnt, 16x128 tiles) SBUF->SBUF --
    off the PE and off HBM.  Only s1->s1T stays on the PE (64-col blocks
    don't meet the XBAR 128-col constraint).
  * bv = s1 @ (s2.T @ c) avoids the [CL,CL] intermediate.
  * the out[:, :, 0:H] = c passthrough block is written straight from the
    c SBUF tiles as soon as they land, independent of all compute.

Sharding: data-parallel over batch, 2 batches per core, no collectives.
"""

import numpy as np
import ml_dtypes
from contextlib import ExitStack

_BF16 = ml_dtypes.bfloat16

import concourse.bass as bass
from concourse import bacc
import concourse.mybir as mybir
import concourse.tile as tile
import concourse.bass_isa as bass_isa
from concourse.masks import make_identity
from concourse.bass_utils import run_bass_kernel_spmd

B, CL, QL, H = 16, 512, 64, 768
NCORES = 8
BPC = B // NCORES  # batches per core
HK = H // 128      # 6 k-tiles over the feature dims
CT = CL // 128     # 4 c-tiles
NH = H // 2        # 384, N per value matmul
NEGB = -1000.0     # additive mask bias; exp(NEGB - max) == 0.0

f32 = mybir.dt.float32
f16 = mybir.dt.float16
bf16 = mybir.dt.bfloat16

Exp = mybir.ActivationFunctionType.Exp
Copy = mybir.ActivationFunctionType.Copy


def _build_nc(precision: int = 1, use_xbar: bool = True) -> bass.Bass:
    nc = bacc.Bacc()
    # all inputs host-packed into SBUF layout: [128 partitions, contiguous]
    cD = nc.declare_dram_parameter("c", [BPC, 128, CT, H], f16, isOutput=False)
    qD = nc.declare_dram_parameter("q", [BPC, QL, H], bf16, isOutput=False)
    cTD = nc.declare_dram_parameter("cT", [BPC, 128, HK, CL], f16,
                                    isOutput=False)
    qTD = nc.declare_dram_parameter("qT", [128, HK, BPC, QL], f16,
                                    isOutput=False)
    W0D = nc.declare_dram_parameter("W0", [128, HK, NH], f16, isOutput=False)
    W1D = nc.declare_dram_parameter("W1", [128, HK, NH + 1], f16,
                                    isOutput=False)
    qbD = nc.declare_dram_parameter("qbias", [BPC, QL], f16, isOutput=False)
    cmD = nc.declare_dram_parameter("cmask", [BPC, QL, CL], f16,
                                    isOutput=False)
    # device computes only the a / c*a / c*bv blocks, tile-major
    outD = nc.declare_dram_parameter("out", [BPC, CT, 128, 3, H], f16,
                                     isOutput=True)

    with tile.TileContext(nc) as tc, ExitStack() as ctx:
        const = ctx.enter_context(tc.tile_pool(name="const", bufs=1))
        wpool = ctx.enter_context(tc.tile_pool(name="wpool", bufs=1))
        perb = ctx.enter_context(tc.tile_pool(name="perb", bufs=2))
        small = ctx.enter_context(tc.tile_pool(name="small", bufs=2))
        outp = ctx.enter_context(tc.tile_pool(name="outp", bufs=3))
        pst = ctx.enter_context(tc.tile_pool(name="pst", bufs=2, space="PSUM"))
        pacc = ctx.enter_context(tc.tile_pool(name="pacc", bufs=4, space="PSUM"))
        ptp = ctx.enter_context(tc.tile_pool(name="ptp", bufs=2, space="PSUM"))

        # --- shared weights + biases ---
        w0_sb = wpool.tile([128, HK, NH], f16)
        w1_sb = wpool.tile([128, HK, NH + 1], f16)
        qbias_sb = wpool.tile([1, BPC * QL], f16)
        nc.scalar.dma_start(out=qbias_sb,
                            in_=qbD[:].rearrange("(o b) q -> o (b q)", o=1))

        # --- qT (host-transposed) ---
        qT2 = wpool.tile([128, HK, BPC, QL], f16)  # [d, k, b, q]
        nc.scalar.dma_start(out=qT2, in_=qTD[:])
        nc.scalar.dma_start(out=w0_sb, in_=W0D[:])
        nc.scalar.dma_start(out=w1_sb, in_=W1D[:])
        ones_row = const.tile([1, CL], f16)
        nc.vector.memset(ones_row, 1.0)
        identb = const.tile([QL, QL], bf16)
        make_identity(nc, identb)
        identf = const.tile([128, 128], f16)
        make_identity(nc, identf)
        ones_col = const.tile([QL, 1], bf16)
        nc.vector.memset(ones_col, 1.0)
        # qq = [q | qc] fused rhs for the a/bv matmuls; q lands via DMA,
        # qc is copied in after the s2 pass
        qqs = []
        for bi in range(BPC):
            qq = perb.tile([QL, 2 * H], bf16, tag="qq")
            nc.sync.dma_start(out=qq[:, 0:H], in_=qD[bi])
            qqs.append(qq)
        cmask_sb = wpool.tile([QL, BPC, CL], f16)

        # --- c natural; passthrough block out; cT (host-transposed) ---
        c_nats, cTs = [], []
        for bi in range(BPC):
            cT = perb.tile([128, HK, CL], f16, tag="cT")
            nc.scalar.dma_start(out=cT, in_=cTD[bi])
            cTs.append(cT)
        nc.scalar.dma_start(out=cmask_sb,
                            in_=cmD[:].rearrange("b q c -> q b c"))
        for bi in range(BPC):
            c_nat = perb.tile([128, CT, H], f16, tag="c_nat")
            nc.scalar.dma_start(out=c_nat, in_=cD[bi])
            c_nats.append(c_nat)

        # --- qWT[h, (b q)] = sum_d W[d,h] qT[d, (b q)], hm-outer waves;
        # qrow = b . q rides as W1's 385th column via a tiny transpose ---
        qwt = wpool.tile([128, HK, BPC * QL], f16)

        def qwt_wave(hms):
            for hm in hms:
                wj = w0_sb if hm < 3 else w1_sb
                ps_w = pacc.tile([128, BPC * QL], f32, tag="acc",
                                 name=f"ps_w{hm}")
                for k in range(HK):
                    nc.tensor.matmul(
                        ps_w,
                        wj[:, k, (hm % 3) * 128:(hm % 3 + 1) * 128],
                        qT2[:, k].rearrange("p b q -> p (b q)"),
                        start=(k == 0), stop=(k == HK - 1))
                nc.vector.tensor_copy(out=qwt[:, hm, :], in_=ps_w)

        qwt_wave(range(0, 3))
        ps_qb = pacc.tile([1, BPC * QL], f32, tag="acc", name="ps_qb")
        for k in range(HK):
            nc.tensor.matmul(ps_qb, w1_sb[:, k, NH:NH + 1],
                             qT2[:, k].rearrange("p b q -> p (b q)"),
                             start=(k == 0), stop=(k == HK - 1))
        qwt_wave(range(3, HK))

        # --- rank-2 bias operands: [qrow;1].T @ [1;cbias] ---
        qrow16 = wpool.tile([1, BPC * QL], f16)
        nc.vector.tensor_add(qrow16, ps_qb, qbias_sb)

        # ---- per-batch pipeline stages ----
        st = [dict() for _ in range(BPC)]

        def stage_logits(bi):
            # biased logits sT[q, c] in PSUM
            ps_st = pst.tile([QL, CL], f32, tag="st",
                              name=f"ps_st{bi}")
            for k in range(HK):
                nc.tensor.matmul(ps_st, qwt[:, k, bi * QL:(bi + 1) * QL],
                                 cTs[bi][:, k], start=(k == 0), stop=False)
            nc.tensor.matmul(ps_st, qrow16[:, bi * QL:(bi + 1) * QL],
                             ones_row, start=False, stop=True)
            st[bi]["ps_st"] = ps_st

        def stage_softmax2(bi):
            # shared exp: e2[q, c] = exp(s - rowmax_q) serves both softmaxes.
            # s2 = e2 / rowsum.  s1[:, c] = e2[:, c] * w / colsum(e2 * w)
            # with w_q = exp(rowmax_q - G), G the global max -- exact, and
            # bf16 e2 never flushes (fp16 would: gaps run beyond 2^-24).
            ps_st = st[bi]["ps_st"]
            nmax2 = small.tile([QL, 1], f32, tag="nmax2")
            nc.vector.reduce_max(nmax2, ps_st, axis=mybir.AxisListType.X,
                                 negate=True)
            # exponent re-centering: e2b = exp(s - rowmax + 44) and
            # w = exp(rowmax - G + 16), so products run at e^(s-G+60) --
            # keeps every per-column dominant term normal in fp32/bf16
            # (the +60 cancels in the a~/D ratio)
            b44 = small.tile([QL, 1], f32, tag="b44")
            nc.scalar.activation(b44, nmax2, Copy, scale=1.0, bias=44.0)
            e2b = small.tile([QL, CL], bf16, tag="e2b")
            nc.scalar.activation(e2b, ps_st, Exp, bias=b44, scale=1.0)
            st[bi]["e2b"] = e2b
            # s2 masks context positions multiplicatively (the c-mask must
            # NOT touch e2b: reference s1 applies no c-mask)
            e2m = small.tile([QL, CL], bf16, tag="e2m")
            nc.vector.tensor_mul(e2m, e2b, cmask_sb[:, bi, :])
            st[bi]["e2m"] = e2m
            sum2 = small.tile([QL, 1], f32, tag="sum2")
            nc.vector.reduce_sum(sum2, e2m, axis=mybir.AxisListType.X)
            r2 = small.tile([QL, 1], f32, tag="r2")
            nc.vector.reciprocal(r2, sum2)
            st[bi]["r2"] = r2
            # w_q = exp(rowmax_q - G); G via gpsimd partition all-reduce
            rm16 = small.tile([QL, 1], f32, tag="rm16")
            nc.scalar.activation(rm16, nmax2, Copy, scale=-1.0, bias=16.0)
            rm32 = small.tile([QL, 1], f32, tag="rm32")
            nc.scalar.activation(rm32, nmax2, Copy, scale=-1.0, bias=32.0)
            g_all = small.tile([QL, 1], f32, tag="g_all")
            nc.gpsimd.partition_all_reduce(g_all, rm16, channels=QL,
                                           reduce_op=bass_isa.ReduceOp.max)
            w32 = small.tile([QL, 1], f32, tag="w32")
            nc.scalar.activation(w32, g_all, Exp, bias=rm32, scale=-1.0)
            # fold the s1 row-weights into the shared lhsT
            e2s = small.tile([QL, CL], bf16, tag="e2s")
            nc.vector.tensor_scalar_mul(e2s, e2b, w32)
            st[bi]["e2s"] = e2s

        def stage_xpose(bi):
            # transpose the unnormalized e2m; 1/sum2 folds into the qc copy
            s2 = small.tile([128, CT, QL], bf16, tag="s2")
            for ci in range(CT):
                tp = ptp.tile([128, QL], bf16, tag="tp")
                nc.tensor.transpose(
                    tp, st[bi]["e2m"][:, ci * 128:(ci + 1) * 128], identb)
                nc.vector.tensor_copy(out=s2[:, ci, :], in_=tp)
            st[bi]["s2"] = s2

        def stage_qc(bi):
            # qc[q, h] = s2.T @ c, written into the back half of qq
            qq = qqs[bi]
            for hf in range(2):
                ps_qc = pacc.tile([QL, NH], f32, tag="acc")
                for ci in range(CT):
                    nc.tensor.matmul(ps_qc, st[bi]["s2"][:, ci, :],
                                     c_nats[bi][:, ci, hf * NH:(hf + 1) * NH],
                                     start=(ci == 0), stop=(ci == CT - 1))
                nc.vector.tensor_scalar_mul(
                    qq[:, H + hf * NH:H + (hf + 1) * NH], ps_qc,
                    st[bi]["r2"])

        def stage_out(bi):
            # [a~ | bv~] = (e2*w).T @ [q | qc]; rows scaled by 1/D with
            # D[c] = sum_q e2[q,c] w_q (tiny PE matmul against ones)
            e2s, qq, c_nat = st[bi]["e2s"], qqs[bi], c_nats[bi]
            for ci in range(CT):
                cs = slice(ci * 128, (ci + 1) * 128)
                ob = outp.tile([128, 3, H], f16, tag="ob")
                a_sb, ca_sb, cbv_sb = ob[:, 0, :], ob[:, 1, :], ob[:, 2, :]
                dps = pacc.tile([128, 1], f32, tag="acc", name="dps")
                nc.tensor.matmul(dps, e2s[:, cs], ones_col, start=True,
                                 stop=True)
                recipD = small.tile([128, 1], f32, tag="recipD")
                nc.vector.reciprocal(recipD, dps)
                ps = []
                for j in range(3):
                    p = pacc.tile([128, 512], f32, tag="acc", name=f"ps{j}")
                    nc.tensor.matmul(p, e2s[:, cs],
                                     qq[:, j * 512:(j + 1) * 512],
                                     start=True, stop=True)
                    ps.append(p)
                # a = ps0[0:512] | ps1[0:256]; bv = ps1[256:512] | ps2[0:512]
                nc.scalar.activation(a_sb[:, 0:512], ps[0], Copy,
                                     scale=recipD)
                nc.scalar.activation(a_sb[:, 512:H], ps[1][:, 0:H - 512],
                                     Copy, scale=recipD)
                nc.vector.tensor_mul(ca_sb, c_nat[:, ci, :], a_sb)
                bv_sb = small.tile([128, H], f16, tag="bv_sb")
                nc.scalar.activation(bv_sb[:, 0:256], ps[1][:, H - 512:512],
                                     Copy, scale=recipD)
                nc.scalar.activation(bv_sb[:, 256:H], ps[2], Copy,
                                     scale=recipD)
                nc.vector.tensor_mul(cbv_sb, c_nat[:, ci, :], bv_sb)
                nc.sync.dma_start(out=outD[bi, ci], in_=ob)

        # interleave the two batches so PE work of one overlaps
        # scalar/vector/DMA work of the other
        stage_logits(0)
        stage_softmax2(0)
        stage_logits(1)
        stage_xpose(0)
        stage_softmax2(1)
        stage_xpose(1)
        stage_qc(0)
        stage_out(0)
        stage_qc(1)
        stage_out(1)

    nc.finalize()
    return nc


_NC_CACHE: dict = {}


def _get_nc(precision: int = 1) -> bass.Bass:
    if precision not in _NC_CACHE:
        _NC_CACHE[precision] = _build_nc(precision)
    return _NC_CACHE[precision]


def _core_inputs(c, q, c_mask, q_mask, W, b, core: int) -> dict:
    sl = slice(core * BPC, (core + 1) * BPC)
    f16n = np.float16
    c16 = np.asarray(c)[sl].astype(f16n)
    q16 = np.ascontiguousarray(np.asarray(q)[sl], dtype=f16n)
    W16 = np.asarray(W).astype(f16n)
    return {
        # c[bi, p, t, h] = c16[bi, 128 t + p, h]
        "c": np.ascontiguousarray(
            c16.reshape(BPC, CT, 128, H).transpose(0, 2, 1, 3)),
        "q": np.ascontiguousarray(np.asarray(q)[sl], dtype=_BF16),
        # cT[bi, p, k, cl] = c16[bi, cl, 128 k + p]
        "cT": np.ascontiguousarray(
            c16.transpose(0, 2, 1).reshape(BPC, HK, 128, CL)
            .transpose(0, 2, 1, 3)),
        # qT[p, k, bi, ql] = q16[bi, ql, 128 k + p]
        "qT": np.ascontiguousarray(
            q16.transpose(0, 2, 1).reshape(BPC, HK, 128, QL)
            .transpose(2, 1, 0, 3)),
        # W0[p, k, h'] = W16[128 k + p, h'];  W1 adds b as column 384
        "W0": np.ascontiguousarray(
            W16[:, 0:NH].reshape(HK, 128, NH)[:, :, :].transpose(1, 0, 2)),
        "W1": np.ascontiguousarray(np.concatenate([
            W16[:, NH:H].reshape(HK, 128, NH),
            np.asarray(b).astype(f16n).reshape(HK, 128, 1)],
            axis=2).transpose(1, 0, 2)),
        "qbias": ((np.asarray(q_mask)[sl].astype(np.float32) - 1.0)
                  * (-NEGB)).astype(f16n),
        # 0/1 context mask pre-broadcast along q (multiplicative, s2 only)
        "cmask": np.broadcast_to(
            np.asarray(c_mask)[sl].astype(f16n)[:, None, :],
            (BPC, QL, CL)).copy(),
    }


def kernel(c, q, c_mask, q_mask, W, b, _trace=False, _precision=1):
    nc = _get_nc(_precision)
    in_maps = [
        _core_inputs(c, q, c_mask, q_mask, W, b, i) for i in range(NCORES)
    ]
    res = run_bass_kernel_spmd(nc, in_maps, core_ids=list(range(NCORES)),
                               trace=_trace)
    out = np.empty((B, CL, 4 * H), dtype=np.float32)
    out[:, :, 0:H] = np.asarray(c, dtype=np.float32)
    for i in range(NCORES):
        # device out: [BPC, CT, 128, 3, H] tile-major -> [BPC, CL, 3H]
        dev = res.results[i]["out"].astype(np.float32)
        dev = dev.reshape(BPC, CT * 128, 3 * H)
        out[i * BPC:(i + 1) * BPC, :, H:] = dev
    if _trace:
        return out, res
    return out


# revision 38
# speedup vs baseline: 1.1795x; 1.1795x over previous
"""BiDAF attention (nn_BertBidafAttention) on 8 TRN2 NeuronCores.

Math (per batch, reference):
    cp = c @ W.T + b            [CL, H]
    s  = cp @ q.T               [CL, QL]
    s1 = softmax_q(s + qmask_bias)      (softmax over q)
    s2 = softmax_c(s + cmask_bias)      (softmax over c)
    a  = s1 @ q                 [CL, H]
    bv = (s1 @ s2.T) @ c = s1 @ (s2.T @ c)
    x  = [c, a, c*a, c*bv]      [CL, 4H]

Implementation (fp16/bf16 end to end, fp32 PSUM accumulation):
  * The host pre-packs every input into its exact SBUF layout (128
    contiguous bytes-per-partition rows) so each load is one
    128-descriptor DMA, pre-transposes cT/qT, and casts to fp16 (10-bit
    mantissa ~ f32r precision).  16-bit matmuls run single-pass at full
    PE rate (fp32 runs two passes).
  * Projection on the q side: qw = W.T @ qT is 75 MF vs 604 MF for c@W.
    b rides as a 385th W column.  Logits sT[q,c] = qw.T @ cT land in
    PSUM with the q-row bias added by a rank-1 matmul, so both softmaxes
    read one fp32 logits tile (free-axis reductions only).
  * A single exp serves both softmaxes: e2 = exp(s - rowmax + 44) in
    bf16 (never flushes).  s2 = (e2 * cmask) / rowsum -- the c-mask is
    multiplicative so the s1 path stays unmasked, and 1/rowsum folds
    into the qc PSUM->SBUF copy.  s1 needs a partition-axis softmax;
    instead of transposing, s1 = e2 * w / D with w = exp(rowmax - G +
    16) (G = global max via gpsimd partition_all_reduce) and
    D[c] = sum_q e2 w, a tiny PE matmul against ones; 1/D folds into
    the output copy/scale ops.  The e^(s-G+60) product centering keeps
    every column's dominant term fp32-normal.
  * bv = s1 @ (s2.T @ c) avoids the [CL,CL] intermediate; a and bv
    share one fused rhs [q | qc] (three N=512 matmuls per c-tile).
  * The out[:, :, 0:H] = c passthrough block is assembled on the host
    (it is an identity copy of an input); the device writes only the
    a / c*a / c*bv blocks, fp16, upcast on the host.

Sharding: data-parallel over batch, 2 batches per core, no collectives.
"""

import numpy as np
import ml_dtypes
from contextlib import ExitStack

_BF16 = ml_dtypes.bfloat16

import concourse.bass as bass
from concourse import bacc
import concourse.mybir as mybir
import concourse.tile as tile
import concourse.bass_isa as bass_isa
from concourse.masks import make_identity
from concourse.bass_utils import run_bass_kernel_spmd

B, CL, QL, H = 16, 512, 64, 768
NCORES = 8
BPC = B // NCORES  # batches per core
HK = H // 128      # 6 k-tiles over the feature dims
CT = CL // 128     # 4 c-tiles
NH = H // 2        # 384, N per value matmul
NEGB = -1000.0     # additive mask bias; exp(NEGB - max) == 0.0

f32 = mybir.dt.float32
f16 = mybir.dt.float16
bf16 = mybir.dt.bfloat16

Exp = mybir.ActivationFunctionType.Exp
Copy = mybir.ActivationFunctionType.Copy


def _build_nc(precision: int = 1) -> bass.Bass:
    nc = bacc.Bacc()
    # all inputs host-packed into SBUF layout: [128 partitions, contiguous]
    cD = nc.declare_dram_parameter("c", [BPC, 128, CT, H], f16, isOutput=False)
    qD = nc.declare_dram_parameter("q", [BPC, QL, H], bf16, isOutput=False)
    cTD = nc.declare_dram_parameter("cT", [BPC, 128, HK, CL], f16,
                                    isOutput=False)
    qTD = nc.declare_dram_parameter("qT", [128, HK, BPC, QL], f16,
                                    isOutput=False)
    W0D = nc.declare_dram_parameter("W0", [128, HK, NH], f16, isOutput=False)
    W1D = nc.declare_dram_parameter("W1", [128, HK, NH + 1], f16,
                                    isOutput=False)
    qbD = nc.declare_dram_parameter("qbias", [BPC, QL], f16, isOutput=False)
    cmD = nc.declare_dram_parameter("cmask", [BPC, QL, CL], f16,
                                    isOutput=False)
    # device computes only the a / c*a / c*bv blocks, tile-major
    outD = nc.declare_dram_parameter("out", [BPC, CT, 128, 3, H], f16,
                                     isOutput=True)

    with tile.TileContext(nc) as tc, ExitStack() as ctx:
        const = ctx.enter_context(tc.tile_pool(name="const", bufs=1))
        wpool = ctx.enter_context(tc.tile_pool(name="wpool", bufs=1))
        perb = ctx.enter_context(tc.tile_pool(name="perb", bufs=2))
        small = ctx.enter_context(tc.tile_pool(name="small", bufs=2))
        outp = ctx.enter_context(tc.tile_pool(name="outp", bufs=4))
        pst = ctx.enter_context(tc.tile_pool(name="pst", bufs=2, space="PSUM"))
        pacc = ctx.enter_context(tc.tile_pool(name="pacc", bufs=4, space="PSUM"))
        ptp = ctx.enter_context(tc.tile_pool(name="ptp", bufs=2, space="PSUM"))

        # --- shared weights + biases ---
        w0_sb = wpool.tile([128, HK, NH], f16)
        w1_sb = wpool.tile([128, HK, NH + 1], f16)
        qbias_sb = wpool.tile([1, BPC * QL], f16)
        nc.scalar.dma_start(out=qbias_sb,
                            in_=qbD[:].rearrange("(o b) q -> o (b q)", o=1))

        # --- qT (host-transposed) ---
        qT2 = wpool.tile([128, HK, BPC, QL], f16)  # [d, k, b, q]
        nc.scalar.dma_start(out=qT2, in_=qTD[:])
        nc.scalar.dma_start(out=w0_sb, in_=W0D[:])
        nc.scalar.dma_start(out=w1_sb, in_=W1D[:])
        ones_row = const.tile([1, CL], f16)
        nc.vector.memset(ones_row, 1.0)
        identb = const.tile([QL, QL], bf16)
        make_identity(nc, identb)
        identf = const.tile([128, 128], f16)
        make_identity(nc, identf)
        ones_col = const.tile([QL, 1], bf16)
        nc.vector.memset(ones_col, 1.0)
        # qq = [q | qc] fused rhs for the a/bv matmuls; q lands via DMA,
        # qc is copied in after the s2 pass
        qqs = []
        for bi in range(BPC):
            qq = perb.tile([QL, 2 * H], bf16, tag="qq")
            nc.sync.dma_start(out=qq[:, 0:H], in_=qD[bi])
            qqs.append(qq)
        cmask_sb = wpool.tile([QL, BPC, CL], f16)

        # --- c natural; passthrough block out; cT (host-transposed) ---
        c_nats, cTs = [], []
        for bi in range(BPC):
            cT = perb.tile([128, HK, CL], f16, tag="cT")
            nc.scalar.dma_start(out=cT, in_=cTD[bi])
            cTs.append(cT)
        nc.scalar.dma_start(out=cmask_sb,
                            in_=cmD[:].rearrange("b q c -> q b c"))
        for bi in range(BPC):
            c_nat = perb.tile([128, CT, H], f16, tag="c_nat")
            nc.scalar.dma_start(out=c_nat, in_=cD[bi])
            c_nats.append(c_nat)

        # --- qWT[h, (b q)] = sum_d W[d,h] qT[d, (b q)], hm-outer waves;
        # qrow = b . q rides as W1's 385th column via a tiny transpose ---
        qwt = wpool.tile([128, HK, BPC * QL], f16)

        def qwt_wave(hms):
            for hm in hms:
                wj = w0_sb if hm < 3 else w1_sb
                ps_w = pacc.tile([128, BPC * QL], f32, tag="acc",
                                 name=f"ps_w{hm}")
                for k in range(HK):
                    nc.tensor.matmul(
                        ps_w,
                        wj[:, k, (hm % 3) * 128:(hm % 3 + 1) * 128],
                        qT2[:, k].rearrange("p b q -> p (b q)"),
                        start=(k == 0), stop=(k == HK - 1))
                nc.vector.tensor_copy(out=qwt[:, hm, :], in_=ps_w)

        qwt_wave(range(0, 3))
        ps_qb = pacc.tile([1, BPC * QL], f32, tag="acc", name="ps_qb")
        for k in range(HK):
            nc.tensor.matmul(ps_qb, w1_sb[:, k, NH:NH + 1],
                             qT2[:, k].rearrange("p b q -> p (b q)"),
                             start=(k == 0), stop=(k == HK - 1))
        qwt_wave(range(3, HK))

        # --- rank-2 bias operands: [qrow;1].T @ [1;cbias] ---
        qrow16 = wpool.tile([1, BPC * QL], f16)
        nc.vector.tensor_add(qrow16, ps_qb, qbias_sb)

        # ---- per-batch pipeline stages ----
        st = [dict() for _ in range(BPC)]

        def stage_logits(bi):
            # biased logits sT[q, c] in PSUM
            ps_st = pst.tile([QL, CL], f32, tag="st",
                              name=f"ps_st{bi}")
            for k in range(HK):
                nc.tensor.matmul(ps_st, qwt[:, k, bi * QL:(bi + 1) * QL],
                                 cTs[bi][:, k], start=(k == 0), stop=False)
            nc.tensor.matmul(ps_st, qrow16[:, bi * QL:(bi + 1) * QL],
                             ones_row, start=False, stop=True)
            st[bi]["ps_st"] = ps_st

        def stage_softmax2(bi):
            # shared exp: e2[q, c] = exp(s - rowmax_q) serves both softmaxes.
            # s2 = e2 / rowsum.  s1[:, c] = e2[:, c] * w / colsum(e2 * w)
            # with w_q = exp(rowmax_q - G), G the global max -- exact, and
            # bf16 e2 never flushes (fp16 would: gaps run beyond 2^-24).
            ps_st = st[bi]["ps_st"]
            nmax2 = small.tile([QL, 1], f32, tag="nmax2")
            nc.vector.reduce_max(nmax2, ps_st, axis=mybir.AxisListType.X,
                                 negate=True)
            # exponent re-centering: e2b = exp(s - rowmax + 44) and
            # w = exp(rowmax - G + 16), so products run at e^(s-G+60) --
            # keeps every per-column dominant term normal in fp32/bf16
            # (the +60 cancels in the a~/D ratio)
            b44 = small.tile([QL, 1], f32, tag="b44")
            nc.scalar.activation(b44, nmax2, Copy, scale=1.0, bias=44.0)
            e2b = small.tile([QL, CL], bf16, tag="e2b")
            nc.scalar.activation(e2b, ps_st, Exp, bias=b44, scale=1.0)
            st[bi]["e2b"] = e2b
            # s2 masks context positions multiplicatively (the c-mask must
            # NOT touch e2b: reference s1 applies no c-mask)
            e2m = small.tile([QL, CL], bf16, tag="e2m")
            nc.vector.tensor_mul(e2m, e2b, cmask_sb[:, bi, :])
            st[bi]["e2m"] = e2m
            sum2 = small.tile([QL, 1], f32, tag="sum2")
            nc.vector.reduce_sum(sum2, e2m, axis=mybir.AxisListType.X)
            r2 = small.tile([QL, 1], f32, tag="r2")
            nc.vector.reciprocal(r2, sum2)
            st[bi]["r2"] = r2
            # w_q = exp(rowmax_q - G); G via gpsimd partition all-reduce
            rm16 = small.tile([QL, 1], f32, tag="rm16")
            nc.scalar.activation(rm16, nmax2, Copy, scale=-1.0, bias=16.0)
            rm32 = small.tile([QL, 1], f32, tag="rm32")
            nc.scalar.activation(rm32, nmax2, Copy, scale=-1.0, bias=32.0)
            g_all = small.tile([QL, 1], f32, tag="g_all")
            nc.gpsimd.partition_all_reduce(g_all, rm16, channels=QL,
                                           reduce_op=bass_isa.ReduceOp.max)
            w32 = small.tile([QL, 1], f32, tag="w32")
            nc.scalar.activation(w32, g_all, Exp, bias=rm32, scale=-1.0)
            # fold the s1 row-weights into the shared lhsT
            e2s = small.tile([QL, CL], bf16, tag="e2s")
            nc.vector.tensor_scalar_mul(e2s, e2b, w32)
            st[bi]["e2s"] = e2s

        def stage_xpose(bi):
            # transpose the unnormalized e2m; 1/sum2 folds into the qc copy
            s2 = small.tile([128, CT, QL], bf16, tag="s2")
            for ci in range(CT):
                tp = ptp.tile([128, QL], bf16, tag="tp")
                nc.tensor.transpose(
                    tp, st[bi]["e2m"][:, ci * 128:(ci + 1) * 128], identb)
                nc.vector.tensor_copy(out=s2[:, ci, :], in_=tp)
            st[bi]["s2"] = s2

        def stage_qc(bi):
            # qc[q, h] = s2.T @ c, written into the back half of qq
            qq = qqs[bi]
            for hf in range(2):
                ps_qc = pacc.tile([QL, NH], f32, tag="acc")
                for ci in range(CT):
                    nc.tensor.matmul(ps_qc, st[bi]["s2"][:, ci, :],
                                     c_nats[bi][:, ci, hf * NH:(hf + 1) * NH],
                                     start=(ci == 0), stop=(ci == CT - 1))
                nc.vector.tensor_scalar_mul(
                    qq[:, H + hf * NH:H + (hf + 1) * NH], ps_qc,
                    st[bi]["r2"])

        def stage_out(bi):
            # [a~ | bv~] = (e2*w).T @ [q | qc]; rows scaled by 1/D with
            # D[c] = sum_q e2[q,c] w_q (tiny PE matmul against ones)
            e2s, qq, c_nat = st[bi]["e2s"], qqs[bi], c_nats[bi]
            for ci in range(CT):
                cs = slice(ci * 128, (ci + 1) * 128)
                ob = outp.tile([128, 3, H], f16, tag="ob")
                a_sb, ca_sb, cbv_sb = ob[:, 0, :], ob[:, 1, :], ob[:, 2, :]
                dps = pacc.tile([128, 1], f32, tag="acc", name="dps")
                nc.tensor.matmul(dps, e2s[:, cs], ones_col, start=True,
                                 stop=True)
                recipD = small.tile([128, 1], f32, tag="recipD")
                nc.vector.reciprocal(recipD, dps)
                ps = []
                for j in range(3):
                    p = pacc.tile([128, 512], f32, tag="acc", name=f"ps{j}")
                    nc.tensor.matmul(p, e2s[:, cs],
                                     qq[:, j * 512:(j + 1) * 512],
                                     start=True, stop=True)
                    ps.append(p)
                # a = ps0[0:512] | ps1[0:256]; bv = ps1[256:512] | ps2[0:512]
                nc.scalar.activation(a_sb[:, 0:512], ps[0], Copy,
                                     scale=recipD)
                nc.scalar.activation(a_sb[:, 512:H], ps[1][:, 0:H - 512],
                                     Copy, scale=recipD)
                nc.vector.tensor_mul(ca_sb, c_nat[:, ci, :], a_sb)
                bv_sb = small.tile([128, H], f16, tag="bv_sb")
                nc.scalar.activation(bv_sb[:, 0:256], ps[1][:, H - 512:512],
                                     Copy, scale=recipD)
                nc.scalar.activation(bv_sb[:, 256:H], ps[2], Copy,
                                     scale=recipD)
                nc.vector.tensor_mul(cbv_sb, c_nat[:, ci, :], bv_sb)
                nc.sync.dma_start(out=outD[bi, ci, :, 0:2, :],
                                  in_=ob[:, 0:2, :])
                nc.sync.dma_start(out=outD[bi, ci, :, 2:3, :],
                                  in_=ob[:, 2:3, :])

        # interleave the two batches so PE work of one overlaps
        # scalar/vector/DMA work of the other
        stage_logits(0)
        stage_softmax2(0)
        stage_logits(1)
        stage_xpose(0)
        stage_softmax2(1)
        stage_xpose(1)
        stage_qc(0)
        stage_out(0)
        stage_qc(1)
        stage_out(1)

    nc.finalize()
    return nc


_NC_CACHE: dict = {}


def _get_nc(precision: int = 1) -> bass.Bass:
    if precision not in _NC_CACHE:
        _NC_CACHE[precision] = _build_nc(precision)
    return _NC_CACHE[precision]


def _core_inputs(c, q, c_mask, q_mask, W, b, core: int) -> dict:
    sl = slice(core * BPC, (core + 1) * BPC)
    f16n = np.float16
    c16 = np.asarray(c)[sl].astype(f16n)
    q16 = np.ascontiguousarray(np.asarray(q)[sl], dtype=f16n)
    W16 = np.asarray(W).astype(f16n)
    return {
        # c[bi, p, t, h] = c16[bi, 128 t + p, h]
        "c": np.ascontiguousarray(
            c16.reshape(BPC, CT, 128, H).transpose(0, 2, 1, 3)),
        "q": np.ascontiguousarray(np.asarray(q)[sl], dtype=_BF16),
        # cT[bi, p, k, cl] = c16[bi, cl, 128 k + p]
        "cT": np.ascontiguousarray(
            c16.transpose(0, 2, 1).reshape(BPC, HK, 128, CL)
            .transpose(0, 2, 1, 3)),
        # qT[p, k, bi, ql] = q16[bi, ql, 128 k + p]
        "qT": np.ascontiguousarray(
            q16.transpose(0, 2, 1).reshape(BPC, HK, 128, QL)
            .transpose(2, 1, 0, 3)),
        # W0[p, k, h'] = W16[128 k + p, h'];  W1 adds b as column 384
        "W0": np.ascontiguousarray(
            W16[:, 0:NH].reshape(HK, 128, NH)[:, :, :].transpose(1, 0, 2)),
        "W1": np.ascontiguousarray(np.concatenate([
            W16[:, NH:H].reshape(HK, 128, NH),
            np.asarray(b).astype(f16n).reshape(HK, 128, 1)],
            axis=2).transpose(1, 0, 2)),
        "qbias": ((np.asarray(q_mask)[sl].astype(np.float32) - 1.0)
                  * (-NEGB)).astype(f16n),
        # 0/1 context mask pre-broadcast along q (multiplicative, s2 only)
        "cmask": np.broadcast_to(
            np.asarray(c_mask)[sl].astype(f16n)[:, None, :],
            (BPC, QL, CL)).copy(),
    }


def kernel(c, q, c_mask, q_mask, W, b, _trace=False, _precision=1):
    nc = _get_nc(_precision)
    in_maps = [
        _core_inputs(c, q, c_mask, q_mask, W, b, i) for i in range(NCORES)
    ]
    res = run_bass_kernel_spmd(nc, in_maps, core_ids=list(range(NCORES)),
                               trace=_trace)
    out = np.empty((B, CL, 4 * H), dtype=np.float32)
    out[:, :, 0:H] = np.asarray(c, dtype=np.float32)
    for i in range(NCORES):
        # device out: [BPC, CT, 128, 3, H] tile-major -> [BPC, CL, 3H]
        dev = res.results[i]["out"].astype(np.float32)
        dev = dev.reshape(BPC, CT * 128, 3 * H)
        out[i * BPC:(i + 1) * BPC, :, H:] = dev
    if _trace:
        return out, res
    return out
